# revision 15
# baseline (speedup 1.0000x reference)
"""Causal self-attention (B=1, T=4096, C=768, H=12, D=64) on 8 TRN2 NeuronCores.

Sharding: 8 cores = 4 head-groups (3 heads each) x 2 sequence-groups.
Core c: heads [3*hg, 3*hg+2] where hg=c//2; handles q-chunks of 256 rows,
global chunk g = 2*j + s (s=c%2, j=0..7) -- interleaving balances the causal
triangle so every core runs an identical instruction stream (SPMD), with the
boundary masks supplied as per-core data.

v3 structure:
- The attention inner loop is ScalarE(exp)-latency-bound (~1.4us per
  4-k-block group). All other PE work (K/V projection segments, Q
  projection, output projection) is fed through a filler queue and emitted
  BETWEEN the S and PV matmuls of attention groups, so it executes inside
  the exp-wait bubbles instead of serializing at slot boundaries.
- V is computed directly in [k, d] layout (x^T block as lhsT, W_v as rhs).
- K^T is stored duplicated on both partition halves; S = K^T @ Q runs as
  two concurrent K=64 row-tiles (tile_position (0,0)/(64,0)).
- softmax normalization happens at the projection OUTPUT, where q sits on
  partitions: denominators are transposed via K=1 matmuls into a [128, 6]
  column set, one cheap reciprocal, then per-partition tensor_scalar
  scaling fused with the 3-head combine (scalar_tensor_tensor).
- exp is computed as exp(s/8)/16 (activation bias -ln 16) so the
  unnormalized y fits fp16; the 1/16 cancels in the normalization.
- V-bias and proj-bias contributions are folded into the host epilogue.
"""
import numpy as np

T, C, H, D = 4096, 768, 12, 64
NH = 3          # heads per core
QC = 256        # q rows per slot
P = 128
NSEG = 8
SEGW = T // NSEG  # 512
# sg column-block c holds k-block 4g + COL2KB[c]; arranged so the two
# concurrent row-tiles write different PSUM banks (cols 0:512 vs 512:1024)
COL2KB = [0, 2, 1, 3]
LN16 = 2.772588722239781

_nc_cache = {}


def split_multi_waits(nc):
    """Walrus here accepts only one sync wait per instruction: hoist extras
    onto standalone InstEventSemaphore instructions on the same engine."""
    import concourse.mybir as mybir
    n_split = 0
    for f in nc.m.functions:
        for bb in f.blocks:
            new_insts = []
            for inst in bb.instructions:
                si = inst.sync_info
                if si is not None and len(si.on_wait) > 1:
                    for w in si.on_wait[:-1]:
                        nop = mybir.InstEventSemaphore(
                            name=nc.get_next_instruction_name(), ins=[], outs=[])
                        nop.engine = inst.engine
                        nop.sync_info = mybir.SyncInfo(on_wait=[w], on_update=[])
                        nc.register_instruction(nop)
                        new_insts.append(nop)
                        n_split += 1
                    si.on_wait = si.on_wait[-1:]
                new_insts.append(inst)
            bb.instructions[:] = new_insts
    return n_split


def build_nc(Tloc=T):
    import concourse.bass as bass
    import concourse.mybir as mybir
    import concourse.tile as tile
    from collections import deque
    from contextlib import ExitStack

    f32 = mybir.dt.float32
    f16 = mybir.dt.float16
    EXP = mybir.ActivationFunctionType.Exp
    ADD = mybir.AluOpType.add
    MUL = mybir.AluOpType.mult

    nslot = Tloc // (2 * QC)     # q-chunks per core (8)
    nkb = Tloc // P              # k 128-blocks (32)
    tq = nslot * QC              # q rows per core (2048)

    nc = bass.Bass(trn_type="TRN2")
    xt16 = nc.dram_tensor("xt16", [C, Tloc], f16, kind="ExternalInput")
    xtq16 = nc.dram_tensor("xtq16", [C, tq], f16, kind="ExternalInput")
    wk16 = nc.dram_tensor("wk16", [C, NH * D], f16, kind="ExternalInput")
    wv16 = nc.dram_tensor("wv16", [C, NH * D], f16, kind="ExternalInput")
    wq16 = nc.dram_tensor("wq16", [C, NH * P], f16, kind="ExternalInput")
    wpj16 = nc.dram_tensor("wpj16", [NH, D, C], f16, kind="ExternalInput")
    biasd = nc.dram_tensor("bias", [P, 8], f32, kind="ExternalInput")
    maskd = nc.dram_tensor("mask", [nslot, P, 1024], f16, kind="ExternalInput")
    out16 = nc.dram_tensor("out16", [tq, C], f16, kind="ExternalOutput")

    with tile.TileContext(nc) as tc, ExitStack() as ctx:
        singles = ctx.enter_context(tc.tile_pool(name="singles", bufs=1))
        xhp = ctx.enter_context(tc.tile_pool(name="xh", bufs=4))
        qtp = ctx.enter_context(tc.tile_pool(name="qt", bufs=2))
        ptp = ctx.enter_context(tc.tile_pool(name="pt", bufs=3))
        dnp = ctx.enter_context(tc.tile_pool(name="dn", bufs=2))
        ytp = ctx.enter_context(tc.tile_pool(name="yt", bufs=2))
        tmp_p = ctx.enter_context(tc.tile_pool(name="tmp", bufs=2))
        ostp = ctx.enter_context(tc.tile_pool(name="ost", bufs=2))
        psg = ctx.enter_context(tc.tile_pool(name="psg", bufs=2, space="PSUM"))
        psy = ctx.enter_context(tc.tile_pool(name="psy", bufs=1, space="PSUM"))
        psm = ctx.enter_context(tc.tile_pool(name="psm", bufs=2, space="PSUM"))

        wk_t = singles.tile([P, 6, NH * D], f16)
        wv_t = singles.tile([P, 6, NH * D], f16)
        wq_t = singles.tile([P, 6, NH * P], f16)
        b_t = singles.tile([P, 8], f32)
        one1 = singles.tile([1, 64], f16)
        kt_t = [singles.tile([P, Tloc], f16, tag=f"kt{h}", name=f"kt{h}")
                for h in range(NH)]
        vp_t = singles.tile([P, nkb, NH, 65], f16)
        mask_t = singles.tile([P, nslot, 1024], f16)
        xtq_t = singles.tile([P, 6, tq], f16)
        wpj_t = []
        for h in range(NH):
            w1 = singles.tile([64, C], f16, tag=f"wpj{h}", name=f"wpj{h}")
            wpj_t.append(w1)

        # ---- prologue DMAs, ordered by when compute first needs them:
        # seg0 K (wk, xh0) -> slot0 Q (wq, xtq) -> seg0 V (wv) -> slot0
        # diagonal mask -> seg1 -> slot0 tail (wpj). masks 2+ prefetch
        # per-slot during the pipeline.
        # xtq arrives in per-slot 256-col batches so slot 0's Q only waits
        # on 0.4MB; later batches prefetch per-slot alongside the masks.
        def xtq_batch(j):
            for c in range(6):
                nc.sync.dma_start(xtq_t[:, c, QC * j:QC * (j + 1)],
                                  xtq16[P * c:P * c + P, QC * j:QC * (j + 1)])

        for c in range(6):
            nc.sync.dma_start(wk_t[:, c], wk16[P * c:P * c + P, :])
        xh0 = xhp.tile([P, 6, SEGW], f16, tag="xh", name="xh")
        for c in range(6):
            nc.sync.dma_start(xh0[:, c], xt16[P * c:P * c + P, 0:SEGW])
        xh1 = xhp.tile([P, 6, SEGW], f16, tag="xh", name="xh")
        for c in range(6):
            nc.sync.dma_start(xh1[:, c], xt16[P * c:P * c + P, SEGW:2 * SEGW])
        for c in range(6):
            nc.sync.dma_start(wq_t[:, c], wq16[P * c:P * c + P, :])
        xtq_batch(0)
        for c in range(6):
            nc.sync.dma_start(wv_t[:, c], wv16[P * c:P * c + P, :])
        nc.sync.dma_start(b_t, biasd[:, :])
        nc.sync.dma_start(mask_t[:, 0], maskd[0])
        xtq_batch(1)
        nc.sync.dma_start(mask_t[:, 1], maskd[1])
        for h in range(NH):
            nc.sync.dma_start(wpj_t[h], wpj16[h])

        nc.vector.memset(one1, 1.0)
        nc.vector.memset(vp_t[:, :, :, 64:65], 1.0)

        xh_tiles = {0: xh0, 1: xh1}

        # ---------- work-item generators ----------
        def seg_items(seg):
            """K/V projection for k-blocks 4seg..4seg+3 as filler closures."""
            def dma_item():
                xh = xh_tiles[seg]
                if seg + 2 < NSEG:
                    t = xhp.tile([P, 6, SEGW], f16, tag="xh", name="xh")
                    for c in range(6):
                        nc.sync.dma_start(
                            t[:, c],
                            xt16[P * c:P * c + P,
                                 SEGW * (seg + 2):SEGW * (seg + 3)])
                    xh_tiles[seg + 2] = t
            cols = slice(SEGW * seg, SEGW * (seg + 1))

            def k_item(m, mw, feats):
                def fn():
                    xh = xh_tiles[seg]
                    ps = psm.tile([P, 512], f32, tag="psm", name="ps")
                    for c in range(6):
                        nc.tensor.matmul(ps[0:mw], wk_t[:, c, P * m:P * m + mw],
                                         xh[:, c], start=(c == 0), stop=(c == 5))
                    for h, rows, bcol in feats:
                        nc.vector.tensor_scalar(
                            kt_t[h][0:64, cols], ps[rows],
                            b_t[rows, bcol:bcol + 1], None, ADD)
                        nc.vector.tensor_copy(kt_t[h][64:P, cols],
                                              kt_t[h][0:64, cols])
                return fn

            def v_item(b):
                def fn():
                    xh = xh_tiles[seg]
                    kb = 4 * seg + b
                    pv = psm.tile([P, 512], f32, tag="psm", name="pv")
                    for c in range(6):
                        nc.tensor.matmul(pv[:, 0:NH * D],
                                         xh[:, c, P * b:P * (b + 1)],
                                         wv_t[:, c], start=(c == 0), stop=(c == 5))
                    nc.vector.tensor_copy(
                        vp_t[:, kb, :, 0:64],
                        pv[:, 0:NH * D].rearrange("p (h d) -> p h d", d=64))
                return fn

            items = [dma_item,
                     k_item(0, 128, [(0, slice(0, 64), 0),
                                     (1, slice(64, 128), 0)]),
                     k_item(1, 64, [(2, slice(0, 64), 1)])]
            items += [v_item(b) for b in range(4)]
            return items

        qt_tiles = {}

        def q_items(j):
            """Q^T (duplicated halves) for slot j, one closure per head."""
            def q_item(h):
                def fn():
                    psq = psm.tile([P, 512], f32, tag="psm", name="psq")
                    for c in range(6):
                        nc.tensor.matmul(psq[:, 0:QC],
                                         wq_t[:, c, P * h:P * (h + 1)],
                                         xtq_t[:, c, QC * j:QC * (j + 1)],
                                         start=(c == 0), stop=(c == 5))
                    qh = qtp.tile([P, QC], f16, tag=f"qt{h}", name=f"qt{h}")
                    nc.vector.tensor_scalar(qh, psq[:, 0:QC],
                                            b_t[:, 2 + h:3 + h], None, ADD)
                    qt_tiles.setdefault(j, {})[h] = qh
                return fn
            return [q_item(h) for h in range(NH)]

        def tail_items(j, yt, rT):
            """Output projection + normalize-combine for slot j."""
            ost = ostp.tile([P, 2, C], f16, tag="ost", name="ost")
            items = []

            def proj_item(qb, n0, nw):
                def fn():
                    pps = []
                    for h in range(NH):
                        pp = psm.tile([P, 512], f32, tag="psm", name="pp")
                        nc.tensor.matmul(pp[:, 0:nw],
                                         yt[h][:, P * qb:P * (qb + 1)],
                                         wpj_t[h][:, n0:n0 + nw],
                                         start=True, stop=True)
                        pps.append(pp)
                        if h == 1:
                            t0 = tmp_p.tile([P, 512], f32, tag="t0", name="t0")
                            nc.vector.tensor_scalar(
                                t0[:, 0:nw], pps[0][:, 0:nw],
                                rT[:, 2 * 0 + qb:2 * 0 + qb + 1], None, MUL)
                            pps[0] = t0
                    t1 = tmp_p.tile([P, 512], f32, tag="t1", name="t1")
                    nc.vector.scalar_tensor_tensor(
                        t1[:, 0:nw], pps[1][:, 0:nw],
                        rT[:, 2 * 1 + qb:2 * 1 + qb + 1], pps[0][:, 0:nw],
                        MUL, ADD)
                    nc.vector.scalar_tensor_tensor(
                        ost[:, qb, n0:n0 + nw], pps[2][:, 0:nw],
                        rT[:, 2 * 2 + qb:2 * 2 + qb + 1], t1[:, 0:nw],
                        MUL, ADD)
                return fn

            def out_item(qb):
                def fn():
                    nc.sync.dma_start(
                        out16[QC * j + P * qb:QC * j + P * (qb + 1), :],
                        ost[:, qb])
                return fn

            for qb in range(2):
                for (n0, nw) in [(0, 512), (512, 256)]:
                    items.append(proj_item(qb, n0, nw))
                items.append(out_item(qb))
            return items

        # ---------- filler queues ----------
        segq = deque()   # entries (seg, fn); seg s MUST be emitted before
        genq = deque()   # slot s's attention reads kt/vp for its blocks
        tailq = deque()

        def drain_one():
            if tailq:
                tailq.popleft()()
            elif genq:
                genq.popleft()()
            elif segq:
                segq.popleft()[1]()
            else:
                return False
            return True

        def drain_all(q):
            while q:
                q.popleft()()

        # ---------- main pipeline ----------
        def drain_seg_upto(jj):
            # correctness: diagonal group of slot jj reads kt/vp of seg jj
            while segq and segq[0][0] <= jj:
                segq.popleft()[1]()

        for it in seg_items(0):
            it()
        for it in seg_items(1):
            it()
        for it in q_items(0):
            it()
        segq.extend((2, it) for it in seg_items(2))

        for j in range(nslot):
            if j + 3 < NSEG:
                segq.extend((j + 3, it) for it in seg_items(j + 3))
            if j + 1 < nslot:
                genq.extend(q_items(j + 1))
            if j + 2 < nslot:
                nc.sync.dma_start(mask_t[:, j + 2], maskd[j + 2])
                xtq_batch(j + 2)

            qt = qt_tiles.pop(j)
            mt = mask_t[:, j]
            yA = psy.tile([65, 512], f32, tag="ya", name="ya")
            yB = psy.tile([65, 512], f32, tag="yb", name="yb")
            yac = [yA[:, 0:QC], yA[:, QC:2 * QC], yB[:, 0:QC]]
            den3 = dnp.tile([1, NH * QC], f16, tag="den", name="den")
            yt = []

            def s_group(h, g):
                if g == j:
                    drain_seg_upto(j)
                sg = psg.tile([P, 1024], f32, tag="sg", name="sg")
                for kb_off, base, tp, c0 in ((0, 0, (0, 0), 0),
                                             (1, 64, (64, 0), 512),
                                             (2, 0, (0, 0), 256),
                                             (3, 64, (64, 0), 768)):
                    kb = 4 * g + kb_off
                    nc.tensor.matmul(sg[:, c0:c0 + QC],
                                     kt_t[h][base:base + 64,
                                             P * kb:P * (kb + 1)],
                                     qt[h][base:base + 64],
                                     start=True, stop=True, tile_position=tp)
                return sg

            for h in range(NH):
                sg_cur = s_group(h, 0)
                for g in range(j + 1):
                    sg_next = s_group(h, g + 1) if g < j else None
                    pt = ptp.tile([P, 1024], f16, tag="pt", name="pt")
                    nc.scalar.activation(pt, sg_cur, EXP,
                                         bias=b_t[:, 7:8], scale=0.125)
                    if g == j:
                        nc.vector.tensor_mul(pt, pt, mt)
                    drain_one()
                    if segq or len(tailq) + len(genq) > 10:
                        drain_one()
                    for c in range(4):
                        kb = 4 * g + COL2KB[c]
                        nc.tensor.matmul(yac[h], vp_t[:, kb, h],
                                         pt[:, QC * c:QC * (c + 1)],
                                         start=(g == 0 and c == 0),
                                         stop=(g == j and c == 3))
                    sg_cur = sg_next
                # unnormalized y + denominator row out of PSUM immediately
                # so the single psy buffer recycles with a short chain
                nc.vector.tensor_copy(den3[:, QC * h:QC * (h + 1)],
                                      yac[h][64:65])
                yh = ytp.tile([64, QC], f16, tag=f"yt{h}", name=f"yt{h}")
                nc.vector.tensor_copy(yh, yac[h][0:64])
                yt.append(yh)

            # denominators -> [128, 6] columns via K=1 matmuls, one recip
            dT = psm.tile([P, 512], f32, tag="psm", name="dT")
            for h in range(NH):
                for qb in range(2):
                    nc.tensor.matmul(
                        dT[:, 2 * h + qb:2 * h + qb + 1],
                        den3[:, QC * h + P * qb:QC * h + P * (qb + 1)],
                        one1[:, 0:1], start=True, stop=True)
            rT = dnp.tile([P, 8], f32, tag="rT", name="rT")
            nc.vector.reciprocal(rT[:, 0:6], dT[:, 0:6])

            # before the next slot's attention starts, its Q must be ready
            drain_all(genq)
            tailq.extend(tail_items(j, yt, rT))

        drain_all(tailq)
        drain_all(genq)
        while segq:
            segq.popleft()[1]()

    split_multi_waits(nc)
    return nc


def make_in_maps(x, W_qkv, b_qkv, W_proj, Tloc=T):
    """Shard the full inputs into the 8 per-core input maps."""
    nslot = Tloc // (2 * QC)
    xT = np.ascontiguousarray(x.reshape(Tloc, C).T).astype(np.float32)
    xT16 = xT.astype(np.float16)

    kk = np.arange(P)
    qq = np.arange(QC)
    in_maps = []
    for core in range(8):
        hg, s = core // 2, core % 2
        heads = [3 * hg + i for i in range(NH)]
        wk_c = np.concatenate(
            [W_qkv[:, C + 64 * h:C + 64 * h + 64] for h in heads], axis=1)
        wv_c = np.concatenate(
            [W_qkv[:, 2 * C + 64 * h:2 * C + 64 * h + 64] for h in heads], axis=1)
        wq_c = np.concatenate(
            [np.tile(W_qkv[:, 64 * h:64 * h + 64], (1, 2)) for h in heads], axis=1)
        wpj_c = np.stack([W_proj[64 * h:64 * h + 64, :] for h in heads])

        bias_c = np.zeros((P, 8), np.float32)
        # col 0: b_k heads 0|1, col 1: b_k head 2
        bias_c[0:64, 0] = b_qkv[C + 64 * heads[0]:C + 64 * heads[0] + 64]
        bias_c[64:P, 0] = b_qkv[C + 64 * heads[1]:C + 64 * heads[1] + 64]
        bias_c[0:64, 1] = b_qkv[C + 64 * heads[2]:C + 64 * heads[2] + 64]
        bias_c[:, 7] = -LN16
        # cols 2-4: b_q per head, duplicated halves
        for hi_, h in enumerate(heads):
            bias_c[0:64, 2 + hi_] = b_qkv[64 * h:64 * h + 64]
            bias_c[64:P, 2 + hi_] = b_qkv[64 * h:64 * h + 64]

        qcols = np.concatenate(
            [np.arange(QC * (2 * j + s), QC * (2 * j + s) + QC)
             for j in range(nslot)])
        xtq_16 = np.ascontiguousarray(xT16[:, qcols])

        mask_c = np.zeros((nslot, P, 1024), np.float32)
        for j in range(nslot):
            q0 = QC * (2 * j + s)
            for c in range(4):
                k0 = P * (4 * j + COL2KB[c])
                mask_c[j, :, QC * c:QC * (c + 1)] = (
                    (k0 + kk[:, None]) <= (q0 + qq[None, :]))

        in_maps.append({
            "xt16": xT16, "xtq16": xtq_16,
            "wk16": np.ascontiguousarray(wk_c).astype(np.float16),
            "wv16": np.ascontiguousarray(wv_c).astype(np.float16),
            "wq16": np.ascontiguousarray(wq_c).astype(np.float16),
            "wpj16": np.ascontiguousarray(wpj_c).astype(np.float16),
            "bias": bias_c, "mask": mask_c.astype(np.float16),
        })
    return in_maps


def unshard(results, b_qkv, W_proj, b_proj, Tloc=T):
    nslot = Tloc // (2 * QC)
    out = np.zeros((Tloc, C), np.float64)
    for core in range(8):
        s = core % 2
        r = results[core]["out16"].astype(np.float64)
        for j in range(nslot):
            g0 = QC * (2 * j + s)
            out[g0:g0 + QC] += r[QC * j:QC * (j + 1)]
    # V-bias contribution (y += b_v before proj) folded into the epilogue:
    # b_v @ W_proj summed over all heads, plus the proj bias itself.
    bv = b_qkv[2 * C:3 * C].astype(np.float64)
    out += bv @ W_proj.astype(np.float64) + b_proj.astype(np.float64)
    return out.astype(np.float32).reshape(1, Tloc, C)


_last_result = {}


def kernel(x, mask, W_qkv, b_qkv, W_proj, b_proj):
    from concourse.bass_utils import run_bass_kernel_spmd
    x = np.asarray(x, np.float32)
    W_qkv = np.asarray(W_qkv, np.float32)
    b_qkv = np.asarray(b_qkv, np.float32)
    W_proj = np.asarray(W_proj, np.float32)
    b_proj = np.asarray(b_proj, np.float32)

    if "nc" not in _nc_cache:
        _nc_cache["nc"] = build_nc(T)
    nc = _nc_cache["nc"]
    in_maps = make_in_maps(x, W_qkv, b_qkv, W_proj, T)
    import os
    kwargs = {}
    if os.environ.get("BASS_KERNEL_TRACE"):
        kwargs = dict(trace=True, trace_cores=list(range(8)))
    res = run_bass_kernel_spmd(nc, in_maps, core_ids=list(range(8)), **kwargs)
    _last_result["res"] = res
    return unshard([r for r in res.results], b_qkv, W_proj, b_proj, T)


# revision 16
# speedup vs baseline: 1.0443x; 1.0443x over previous
"""Causal self-attention (B=1, T=4096, C=768, H=12, D=64) on 8 TRN2 NeuronCores.

Sharding: 8 cores = 4 head-groups (3 heads each) x 2 sequence-groups.
Core c: heads [3*hg, 3*hg+2] where hg=c//2; handles q-chunks of 256 rows,
global chunk g = 2*j + s (s=c%2, j=0..7) -- interleaving balances the causal
triangle so every core runs an identical instruction stream (SPMD), with the
boundary masks supplied as per-core data.

v3 structure:
- The attention inner loop is ScalarE(exp)-latency-bound (~1.4us per
  4-k-block group). All other PE work (K/V projection segments, Q
  projection, output projection) is fed through a filler queue and emitted
  BETWEEN the S and PV matmuls of attention groups, so it executes inside
  the exp-wait bubbles instead of serializing at slot boundaries.
- V is computed directly in [k, d] layout (x^T block as lhsT, W_v as rhs).
- K^T is stored duplicated on both partition halves; S = K^T @ Q runs as
  two concurrent K=64 row-tiles (tile_position (0,0)/(64,0)).
- softmax normalization happens at the projection OUTPUT, where q sits on
  partitions: denominators are transposed via K=1 matmuls into a [128, 6]
  column set, one cheap reciprocal, then per-partition tensor_scalar
  scaling fused with the 3-head combine (scalar_tensor_tensor).
- exp is computed as exp(s/8)/16 (activation bias -ln 16) so the
  unnormalized y fits fp16; the 1/16 cancels in the normalization.
- V-bias and proj-bias contributions are folded into the host epilogue.
"""
import numpy as np

T, C, H, D = 4096, 768, 12, 64
NH = 3          # heads per core
QC = 256        # q rows per slot
P = 128
NSEG = 8
SEGW = T // NSEG  # 512
# sg column-block c holds k-block 4g + COL2KB[c]; arranged so the two
# concurrent row-tiles write different PSUM banks (cols 0:512 vs 512:1024)
COL2KB = [0, 2, 1, 3]
LN16 = 2.772588722239781

_nc_cache = {}


def split_multi_waits(nc):
    """Walrus here accepts only one sync wait per instruction: hoist extras
    onto standalone InstEventSemaphore instructions on the same engine."""
    import concourse.mybir as mybir
    n_split = 0
    for f in nc.m.functions:
        for bb in f.blocks:
            new_insts = []
            for inst in bb.instructions:
                si = inst.sync_info
                if si is not None and len(si.on_wait) > 1:
                    for w in si.on_wait[:-1]:
                        nop = mybir.InstEventSemaphore(
                            name=nc.get_next_instruction_name(), ins=[], outs=[])
                        nop.engine = inst.engine
                        nop.sync_info = mybir.SyncInfo(on_wait=[w], on_update=[])
                        nc.register_instruction(nop)
                        new_insts.append(nop)
                        n_split += 1
                    si.on_wait = si.on_wait[-1:]
                new_insts.append(inst)
            bb.instructions[:] = new_insts
    return n_split


def build_nc(Tloc=T):
    import concourse.bass as bass
    import concourse.mybir as mybir
    import concourse.tile as tile
    from collections import deque
    from contextlib import ExitStack

    f32 = mybir.dt.float32
    f16 = mybir.dt.float16
    EXP = mybir.ActivationFunctionType.Exp
    ADD = mybir.AluOpType.add
    MUL = mybir.AluOpType.mult

    nslot = Tloc // (2 * QC)     # q-chunks per core (8)
    nkb = Tloc // P              # k 128-blocks (32)
    tq = nslot * QC              # q rows per core (2048)

    nc = bass.Bass(trn_type="TRN2")
    xt16 = nc.dram_tensor("xt16", [C, Tloc], f16, kind="ExternalInput")
    xtq16 = nc.dram_tensor("xtq16", [C, tq], f16, kind="ExternalInput")
    wk16 = nc.dram_tensor("wk16", [C, NH * D], f16, kind="ExternalInput")
    wv16 = nc.dram_tensor("wv16", [C, NH * D], f16, kind="ExternalInput")
    wq16 = nc.dram_tensor("wq16", [C, NH * P], f16, kind="ExternalInput")
    wpj16 = nc.dram_tensor("wpj16", [NH, D, C], f16, kind="ExternalInput")
    biasd = nc.dram_tensor("bias", [P, 8], f32, kind="ExternalInput")
    maskd = nc.dram_tensor("mask", [nslot, P, 1024], f16, kind="ExternalInput")
    out16 = nc.dram_tensor("out16", [tq, C], f16, kind="ExternalOutput")

    with tile.TileContext(nc) as tc, ExitStack() as ctx:
        singles = ctx.enter_context(tc.tile_pool(name="singles", bufs=1))
        xhp = ctx.enter_context(tc.tile_pool(name="xh", bufs=4))
        qtp = ctx.enter_context(tc.tile_pool(name="qt", bufs=2))
        ptp = ctx.enter_context(tc.tile_pool(name="pt", bufs=3))
        dnp = ctx.enter_context(tc.tile_pool(name="dn", bufs=2))
        ytp = ctx.enter_context(tc.tile_pool(name="yt", bufs=2))
        tmp_p = ctx.enter_context(tc.tile_pool(name="tmp", bufs=2))
        ostp = ctx.enter_context(tc.tile_pool(name="ost", bufs=2))
        psg = ctx.enter_context(tc.tile_pool(name="psg", bufs=2, space="PSUM"))
        psy = ctx.enter_context(tc.tile_pool(name="psy", bufs=1, space="PSUM"))
        psm = ctx.enter_context(tc.tile_pool(name="psm", bufs=2, space="PSUM"))

        wk_t = singles.tile([P, 6, NH * D], f16)
        wv_t = singles.tile([P, 6, NH * D], f16)
        wq_t = singles.tile([P, 6, NH * P], f16)
        b_t = singles.tile([P, 8], f32)
        one1 = singles.tile([1, 64], f16)
        kt_t = [singles.tile([P, Tloc], f16, tag=f"kt{h}", name=f"kt{h}")
                for h in range(NH)]
        vp_t = singles.tile([P, nkb, NH, 65], f16)
        mask_t = singles.tile([P, nslot, 1024], f16)
        xtq_t = singles.tile([P, 6, tq], f16)
        wpj_t = []
        for h in range(NH):
            w1 = singles.tile([64, C], f16, tag=f"wpj{h}", name=f"wpj{h}")
            wpj_t.append(w1)

        # ---- prologue DMAs, ordered by when compute first needs them:
        # seg0 K (wk, xh0) -> slot0 Q (wq, xtq) -> seg0 V (wv) -> slot0
        # diagonal mask -> seg1 -> slot0 tail (wpj). masks 2+ prefetch
        # per-slot during the pipeline.
        # xtq arrives in per-slot 256-col batches so slot 0's Q only waits
        # on 0.4MB; later batches prefetch per-slot alongside the masks.
        xtq_src = xtq16.rearrange("(c p) t -> p c t", p=P)
        xt_src = xt16.rearrange("(c p) t -> p c t", p=P)

        def xtq_batch(j):
            nc.sync.dma_start(xtq_t[:, :, QC * j:QC * (j + 1)],
                              xtq_src[:, :, QC * j:QC * (j + 1)])

        nc.sync.dma_start(wk_t, wk16.rearrange("(c p) f -> p c f", p=P))
        xh0 = xhp.tile([P, 6, SEGW], f16, tag="xh", name="xh")
        nc.sync.dma_start(xh0, xt_src[:, :, 0:SEGW])
        xh1 = xhp.tile([P, 6, SEGW], f16, tag="xh", name="xh")
        nc.sync.dma_start(xh1, xt_src[:, :, SEGW:2 * SEGW])
        nc.sync.dma_start(wq_t, wq16.rearrange("(c p) f -> p c f", p=P))
        xtq_batch(0)
        nc.sync.dma_start(wv_t, wv16.rearrange("(c p) f -> p c f", p=P))
        nc.sync.dma_start(b_t, biasd[:, :])
        nc.sync.dma_start(mask_t[:, 0], maskd[0])
        xtq_batch(1)
        nc.sync.dma_start(mask_t[:, 1], maskd[1])
        for h in range(NH):
            nc.sync.dma_start(wpj_t[h], wpj16[h])

        nc.vector.memset(one1, 1.0)
        nc.vector.memset(vp_t[:, :, :, 64:65], 1.0)

        xh_tiles = {0: xh0, 1: xh1}

        # ---------- work-item generators ----------
        def seg_items(seg):
            """K/V projection for k-blocks 4seg..4seg+3 as filler closures."""
            def dma_item():
                xh = xh_tiles[seg]
                if seg + 2 < NSEG:
                    t = xhp.tile([P, 6, SEGW], f16, tag="xh", name="xh")
                    nc.sync.dma_start(
                        t, xt_src[:, :, SEGW * (seg + 2):SEGW * (seg + 3)])
                    xh_tiles[seg + 2] = t
            cols = slice(SEGW * seg, SEGW * (seg + 1))

            def k_item(m, mw, feats):
                def fn():
                    xh = xh_tiles[seg]
                    ps = psm.tile([P, 512], f32, tag="psm", name="ps")
                    for c in range(6):
                        nc.tensor.matmul(ps[0:mw], wk_t[:, c, P * m:P * m + mw],
                                         xh[:, c], start=(c == 0), stop=(c == 5))
                    for h, rows, bcol in feats:
                        nc.vector.tensor_scalar(
                            kt_t[h][0:64, cols], ps[rows],
                            b_t[rows, bcol:bcol + 1], None, ADD)
                        nc.vector.tensor_copy(kt_t[h][64:P, cols],
                                              kt_t[h][0:64, cols])
                return fn

            def v_item(b):
                def fn():
                    xh = xh_tiles[seg]
                    kb = 4 * seg + b
                    pv = psm.tile([P, 512], f32, tag="psm", name="pv")
                    for c in range(6):
                        nc.tensor.matmul(pv[:, 0:NH * D],
                                         xh[:, c, P * b:P * (b + 1)],
                                         wv_t[:, c], start=(c == 0), stop=(c == 5))
                    nc.vector.tensor_copy(
                        vp_t[:, kb, :, 0:64],
                        pv[:, 0:NH * D].rearrange("p (h d) -> p h d", d=64))
                return fn

            items = [dma_item,
                     k_item(0, 128, [(0, slice(0, 64), 0),
                                     (1, slice(64, 128), 0)]),
                     k_item(1, 64, [(2, slice(0, 64), 1)])]
            items += [v_item(b) for b in range(4)]
            return items

        qt_tiles = {}

        def q_items(j):
            """Q^T (duplicated halves) for slot j, one closure per head."""
            def q_item(h):
                def fn():
                    psq = psm.tile([P, 512], f32, tag="psm", name="psq")
                    for c in range(6):
                        nc.tensor.matmul(psq[:, 0:QC],
                                         wq_t[:, c, P * h:P * (h + 1)],
                                         xtq_t[:, c, QC * j:QC * (j + 1)],
                                         start=(c == 0), stop=(c == 5))
                    qh = qtp.tile([P, QC], f16, tag=f"qt{h}", name=f"qt{h}")
                    nc.vector.tensor_scalar(qh, psq[:, 0:QC],
                                            b_t[:, 2 + h:3 + h], None, ADD)
                    qt_tiles.setdefault(j, {})[h] = qh
                return fn
            return [q_item(h) for h in range(NH)]

        def tail_items(j, yt, rT):
            """Output projection + normalize-combine for slot j."""
            ost = ostp.tile([P, 2, C], f16, tag="ost", name="ost")
            items = []

            def proj_item(qb, n0, nw):
                def fn():
                    pps = []
                    for h in range(NH):
                        pp = psm.tile([P, 512], f32, tag="psm", name="pp")
                        nc.tensor.matmul(pp[:, 0:nw],
                                         yt[h][:, P * qb:P * (qb + 1)],
                                         wpj_t[h][:, n0:n0 + nw],
                                         start=True, stop=True)
                        pps.append(pp)
                        if h == 1:
                            t0 = tmp_p.tile([P, 512], f32, tag="t0", name="t0")
                            nc.vector.tensor_scalar(
                                t0[:, 0:nw], pps[0][:, 0:nw],
                                rT[:, 2 * 0 + qb:2 * 0 + qb + 1], None, MUL)
                            pps[0] = t0
                    t1 = tmp_p.tile([P, 512], f32, tag="t1", name="t1")
                    nc.vector.scalar_tensor_tensor(
                        t1[:, 0:nw], pps[1][:, 0:nw],
                        rT[:, 2 * 1 + qb:2 * 1 + qb + 1], pps[0][:, 0:nw],
                        MUL, ADD)
                    nc.vector.scalar_tensor_tensor(
                        ost[:, qb, n0:n0 + nw], pps[2][:, 0:nw],
                        rT[:, 2 * 2 + qb:2 * 2 + qb + 1], t1[:, 0:nw],
                        MUL, ADD)
                return fn

            def out_item(qb):
                def fn():
                    nc.sync.dma_start(
                        out16[QC * j + P * qb:QC * j + P * (qb + 1), :],
                        ost[:, qb])
                return fn

            for qb in range(2):
                for (n0, nw) in [(0, 512), (512, 256)]:
                    items.append(proj_item(qb, n0, nw))
                items.append(out_item(qb))
            return items

        # ---------- filler queues ----------
        segq = deque()   # entries (seg, fn); seg s MUST be emitted before
        genq = deque()   # slot s's attention reads kt/vp for its blocks
        tailq = deque()

        def drain_one():
            if tailq:
                tailq.popleft()()
            elif genq:
                genq.popleft()()
            elif segq:
                segq.popleft()[1]()
            else:
                return False
            return True

        def drain_all(q):
            while q:
                q.popleft()()

        # ---------- main pipeline ----------
        def drain_seg_upto(jj):
            # correctness: diagonal group of slot jj reads kt/vp of seg jj
            while segq and segq[0][0] <= jj:
                segq.popleft()[1]()

        for it in seg_items(0):
            it()
        for it in seg_items(1):
            it()
        for it in q_items(0):
            it()
        segq.extend((2, it) for it in seg_items(2))

        for j in range(nslot):
            if j + 3 < NSEG:
                segq.extend((j + 3, it) for it in seg_items(j + 3))
            if j + 1 < nslot:
                genq.extend(q_items(j + 1))
            if j + 2 < nslot:
                nc.sync.dma_start(mask_t[:, j + 2], maskd[j + 2])
                xtq_batch(j + 2)

            qt = qt_tiles.pop(j)
            mt = mask_t[:, j]
            yA = psy.tile([65, 512], f32, tag="ya", name="ya")
            yB = psy.tile([65, 512], f32, tag="yb", name="yb")
            yac = [yA[:, 0:QC], yA[:, QC:2 * QC], yB[:, 0:QC]]
            den3 = dnp.tile([1, NH * QC], f16, tag="den", name="den")
            yt = []

            def s_group(h, g):
                if g == j:
                    drain_seg_upto(j)
                sg = psg.tile([P, 1024], f32, tag="sg", name="sg")
                for kb_off, base, tp, c0 in ((0, 0, (0, 0), 0),
                                             (1, 64, (64, 0), 512),
                                             (2, 0, (0, 0), 256),
                                             (3, 64, (64, 0), 768)):
                    kb = 4 * g + kb_off
                    nc.tensor.matmul(sg[:, c0:c0 + QC],
                                     kt_t[h][base:base + 64,
                                             P * kb:P * (kb + 1)],
                                     qt[h][base:base + 64],
                                     start=True, stop=True, tile_position=tp)
                return sg

            for h in range(NH):
                sg_cur = s_group(h, 0)
                for g in range(j + 1):
                    sg_next = s_group(h, g + 1) if g < j else None
                    pt = ptp.tile([P, 1024], f16, tag="pt", name="pt")
                    nc.scalar.activation(pt, sg_cur, EXP,
                                         bias=b_t[:, 7:8], scale=0.125)
                    if g == j:
                        nc.vector.tensor_mul(pt, pt, mt)
                    drain_one()
                    if segq or len(tailq) + len(genq) > 10:
                        drain_one()
                    for c in range(4):
                        kb = 4 * g + COL2KB[c]
                        nc.tensor.matmul(yac[h], vp_t[:, kb, h],
                                         pt[:, QC * c:QC * (c + 1)],
                                         start=(g == 0 and c == 0),
                                         stop=(g == j and c == 3))
                    sg_cur = sg_next
                # unnormalized y + denominator row out of PSUM immediately
                # so the single psy buffer recycles with a short chain
                nc.vector.tensor_copy(den3[:, QC * h:QC * (h + 1)],
                                      yac[h][64:65])
                yh = ytp.tile([64, QC], f16, tag=f"yt{h}", name=f"yt{h}")
                nc.vector.tensor_copy(yh, yac[h][0:64])
                yt.append(yh)

            # denominators -> [128, 6] columns via K=1 matmuls, one recip
            dT = psm.tile([P, 512], f32, tag="psm", name="dT")
            for h in range(NH):
                for qb in range(2):
                    nc.tensor.matmul(
                        dT[:, 2 * h + qb:2 * h + qb + 1],
                        den3[:, QC * h + P * qb:QC * h + P * (qb + 1)],
                        one1[:, 0:1], start=True, stop=True)
            rT = dnp.tile([P, 8], f32, tag="rT", name="rT")
            nc.vector.reciprocal(rT[:, 0:6], dT[:, 0:6])

            # before the next slot's attention starts, its Q must be ready
            drain_all(genq)
            tailq.extend(tail_items(j, yt, rT))

        drain_all(tailq)
        drain_all(genq)
        while segq:
            segq.popleft()[1]()

    split_multi_waits(nc)
    return nc


def make_in_maps(x, W_qkv, b_qkv, W_proj, Tloc=T):
    """Shard the full inputs into the 8 per-core input maps."""
    nslot = Tloc // (2 * QC)
    xT = np.ascontiguousarray(x.reshape(Tloc, C).T).astype(np.float32)
    xT16 = xT.astype(np.float16)

    kk = np.arange(P)
    qq = np.arange(QC)
    in_maps = []
    for core in range(8):
        hg, s = core // 2, core % 2
        heads = [3 * hg + i for i in range(NH)]
        wk_c = np.concatenate(
            [W_qkv[:, C + 64 * h:C + 64 * h + 64] for h in heads], axis=1)
        wv_c = np.concatenate(
            [W_qkv[:, 2 * C + 64 * h:2 * C + 64 * h + 64] for h in heads], axis=1)
        wq_c = np.concatenate(
            [np.tile(W_qkv[:, 64 * h:64 * h + 64], (1, 2)) for h in heads], axis=1)
        wpj_c = np.stack([W_proj[64 * h:64 * h + 64, :] for h in heads])

        bias_c = np.zeros((P, 8), np.float32)
        # col 0: b_k heads 0|1, col 1: b_k head 2
        bias_c[0:64, 0] = b_qkv[C + 64 * heads[0]:C + 64 * heads[0] + 64]
        bias_c[64:P, 0] = b_qkv[C + 64 * heads[1]:C + 64 * heads[1] + 64]
        bias_c[0:64, 1] = b_qkv[C + 64 * heads[2]:C + 64 * heads[2] + 64]
        bias_c[:, 7] = -LN16
        # cols 2-4: b_q per head, duplicated halves
        for hi_, h in enumerate(heads):
            bias_c[0:64, 2 + hi_] = b_qkv[64 * h:64 * h + 64]
            bias_c[64:P, 2 + hi_] = b_qkv[64 * h:64 * h + 64]

        qcols = np.concatenate(
            [np.arange(QC * (2 * j + s), QC * (2 * j + s) + QC)
             for j in range(nslot)])
        xtq_16 = np.ascontiguousarray(xT16[:, qcols])

        mask_c = np.zeros((nslot, P, 1024), np.float32)
        for j in range(nslot):
            q0 = QC * (2 * j + s)
            for c in range(4):
                k0 = P * (4 * j + COL2KB[c])
                mask_c[j, :, QC * c:QC * (c + 1)] = (
                    (k0 + kk[:, None]) <= (q0 + qq[None, :]))

        in_maps.append({
            "xt16": xT16, "xtq16": xtq_16,
            "wk16": np.ascontiguousarray(wk_c).astype(np.float16),
            "wv16": np.ascontiguousarray(wv_c).astype(np.float16),
            "wq16": np.ascontiguousarray(wq_c).astype(np.float16),
            "wpj16": np.ascontiguousarray(wpj_c).astype(np.float16),
            "bias": bias_c, "mask": mask_c.astype(np.float16),
        })
    return in_maps


def unshard(results, b_qkv, W_proj, b_proj, Tloc=T):
    nslot = Tloc // (2 * QC)
    out = np.zeros((Tloc, C), np.float64)
    for core in range(8):
        s = core % 2
        r = results[core]["out16"].astype(np.float64)
        for j in range(nslot):
            g0 = QC * (2 * j + s)
            out[g0:g0 + QC] += r[QC * j:QC * (j + 1)]
    # V-bias contribution (y += b_v before proj) folded into the epilogue:
    # b_v @ W_proj summed over all heads, plus the proj bias itself.
    bv = b_qkv[2 * C:3 * C].astype(np.float64)
    out += bv @ W_proj.astype(np.float64) + b_proj.astype(np.float64)
    return out.astype(np.float32).reshape(1, Tloc, C)


_last_result = {}


def kernel(x, mask, W_qkv, b_qkv, W_proj, b_proj):
    from concourse.bass_utils import run_bass_kernel_spmd
    x = np.asarray(x, np.float32)
    W_qkv = np.asarray(W_qkv, np.float32)
    b_qkv = np.asarray(b_qkv, np.float32)
    W_proj = np.asarray(W_proj, np.float32)
    b_proj = np.asarray(b_proj, np.float32)

    if "nc" not in _nc_cache:
        _nc_cache["nc"] = build_nc(T)
    nc = _nc_cache["nc"]
    in_maps = make_in_maps(x, W_qkv, b_qkv, W_proj, T)
    import os
    kwargs = {}
    if os.environ.get("BASS_KERNEL_TRACE"):
        kwargs = dict(trace=True, trace_cores=list(range(8)))
    res = run_bass_kernel_spmd(nc, in_maps, core_ids=list(range(8)), **kwargs)
    _last_result["res"] = res
    return unshard([r for r in res.results], b_qkv, W_proj, b_proj, T)
